# revision 3
# baseline (speedup 1.0000x reference)
"""FP8-style block-dequant linear: y = x @ (weight * block_scales).T

Full-input contract: kernel(x, weight, weight_scale_inv) -> y [32, 18432] f32.

Strategy (column-parallel over 8 NeuronCores):
  - Shard weight rows (out_features) across cores: each core owns
    O_LOC = 18432/8 = 2304 rows -> computes y[:, c*2304:(c+1)*2304].
  - Host-side layout prep (no arithmetic): per-core transposed weight
    W^T [7168, 2304] so the contraction dim (in_features) lands on SBUF
    partitions with large contiguous DMA lines; x packed into its SBUF
    tile layout; scales replicated across partitions.
  - On-device per core: for each of 56 k-tiles, DMA W^T tile
    [128, 2304], dequant-scale it on DVE (block scale per 128-column
    group), then accumulate matmuls into PSUM with the x^T tile
    [128, 32] as the stationary operand.  Exact fp32 end to end.
"""

import numpy as np

M = 32
I = 7168
O = 18432
NCORES = 8
O_LOC = O // NCORES  # 2304
BLK = 128
IB = I // BLK  # 56 k-tiles
OBL = O_LOC // BLK  # 18 block-columns per core

_CACHE = {}


def _build_nc():
    import concourse.mybir as mybir
    from concourse import bacc
    from concourse.tile import TileContext

    f32 = mybir.dt.float32
    nc = bacc.Bacc()
    wt = nc.declare_dram_parameter("wt", [I, O_LOC], f32, isOutput=False)
    xp = nc.declare_dram_parameter("xp", [BLK, IB * M], f32, isOutput=False)
    ss = nc.declare_dram_parameter("ss", [BLK, IB * OBL], f32, isOutput=False)
    y = nc.declare_dram_parameter("y", [M, O_LOC], f32, isOutput=True)

    with TileContext(nc) as tc:
        with (
            tc.tile_pool(name="consts", bufs=1) as consts,
            tc.tile_pool(name="wp", bufs=3) as wp,
            tc.tile_pool(name="wsp", bufs=3) as wsp,
            tc.tile_pool(name="pp", bufs=1, space="PSUM") as pp,
            tc.tile_pool(name="op", bufs=1) as op,
        ):
            xs = consts.tile([BLK, IB * M], f32)
            nc.sync.dma_start(out=xs, in_=xp[:, :])
            sc = consts.tile([BLK, IB * OBL], f32)
            nc.sync.dma_start(out=sc, in_=ss[:, :])

            ps = pp.tile([M, O_LOC], f32)

            for ib in range(IB):
                w = wp.tile([BLK, O_LOC], f32)
                nc.sync.dma_start(out=w, in_=wt[ib * BLK : (ib + 1) * BLK, :])
                ws = wsp.tile([BLK, O_LOC], f32)
                nc.vector.tensor_mul(
                    out=ws.rearrange("p (ob oc) -> p ob oc", oc=BLK),
                    in0=w.rearrange("p (ob oc) -> p ob oc", oc=BLK),
                    in1=sc[:, ib * OBL : (ib + 1) * OBL].broadcast_to(
                        (BLK, OBL, BLK)
                    ),
                )
                for lo in range(0, O_LOC, 512):
                    hi = min(lo + 512, O_LOC)
                    nc.tensor.matmul(
                        ps[:, lo:hi],
                        xs[:, ib * M : (ib + 1) * M],
                        ws[:, lo:hi],
                        start=(ib == 0),
                        stop=(ib == IB - 1),
                    )

            ysb = op.tile([M, O_LOC], f32)
            nc.vector.tensor_copy(out=ysb, in_=ps)
            nc.sync.dma_start(out=y[:, :], in_=ysb)
    nc.compile()
    return nc


def get_nc():
    if "nc" not in _CACHE:
        _CACHE["nc"] = _build_nc()
    return _CACHE["nc"]


def make_in_maps(x, weight, weight_scale_inv):
    """Host-side shard + layout prep (pure data movement, no arithmetic)."""
    x = np.ascontiguousarray(x, dtype=np.float32)
    weight = np.ascontiguousarray(weight, dtype=np.float32)
    s = np.ascontiguousarray(weight_scale_inv, dtype=np.float32)

    # x packed: xp[p, ib*M + m] = x[m, ib*BLK + p]
    xp = np.ascontiguousarray(
        x.reshape(M, IB, BLK).transpose(2, 1, 0).reshape(BLK, IB * M)
    )

    in_maps = []
    for c in range(NCORES):
        w_c = weight[c * O_LOC : (c + 1) * O_LOC, :]  # [O_LOC, I]
        wt_c = np.ascontiguousarray(w_c.T)  # [I, O_LOC]
        s_c = s[c * OBL : (c + 1) * OBL, :]  # [OBL, IB]
        ss_flat = np.ascontiguousarray(s_c.T).reshape(1, IB * OBL)
        ss_c = np.ascontiguousarray(np.broadcast_to(ss_flat, (BLK, IB * OBL)))
        in_maps.append({"wt": wt_c, "xp": xp, "ss": ss_c})
    return in_maps


def kernel(x, weight, weight_scale_inv):
    from concourse.bass_utils import run_bass_kernel_spmd

    nc = get_nc()
    in_maps = make_in_maps(x, weight, weight_scale_inv)
    res = run_bass_kernel_spmd(nc, in_maps, list(range(NCORES)))
    outs = [res.results[c]["y"] for c in range(NCORES)]
    return np.ascontiguousarray(np.concatenate(outs, axis=1), dtype=np.float32)


# revision 5
# speedup vs baseline: 148.2820x; 148.2820x over previous
"""FP8-style block-dequant linear: y = x @ (weight * block_scales).T

Full-input contract: kernel(x, weight, weight_scale_inv) -> y [32, 18432] f32.

Strategy (column-parallel over 8 NeuronCores):
  - Shard weight rows (out_features) across cores: each core owns
    O_LOC = 18432/8 = 2304 rows -> computes y[:, c*2304:(c+1)*2304].
  - Host-side layout prep (no arithmetic): per-core transposed weight
    W^T [7168, 2304] so the contraction dim (in_features) lands on SBUF
    partitions with large contiguous DMA lines; x packed into its SBUF
    tile layout; scales replicated across partitions.
  - On-device per core: for each of 56 k-tiles, DMA W^T tile
    [128, 2304], dequant-scale it on DVE (block scale per 128-column
    group), then accumulate matmuls into PSUM with the x^T tile
    [128, 32] as the stationary operand.  Exact fp32 end to end.
"""

import numpy as np

M = 32
I = 7168
O = 18432
NCORES = 8
O_LOC = O // NCORES  # 2304
BLK = 128
IB = I // BLK  # 56 k-tiles
OBL = O_LOC // BLK  # 18 block-columns per core

_CACHE = {}


def _build_nc(iters=1):
    import concourse.mybir as mybir
    from concourse import bacc
    from concourse.tile import TileContext

    f32 = mybir.dt.float32
    nc = bacc.Bacc()
    wt = nc.declare_dram_parameter("wt", [I, O_LOC], f32, isOutput=False)
    xp = nc.declare_dram_parameter("xp", [BLK, IB * M], f32, isOutput=False)
    ss = nc.declare_dram_parameter("ss", [BLK, IB * OBL], f32, isOutput=False)
    y = nc.declare_dram_parameter("y", [M, O_LOC], f32, isOutput=True)

    with TileContext(nc) as tc:
        with (
            tc.tile_pool(name="consts", bufs=1) as consts,
            tc.tile_pool(name="wp", bufs=3) as wp,
            tc.tile_pool(name="wsp", bufs=3) as wsp,
            tc.tile_pool(name="pp", bufs=1, space="PSUM") as pp,
            tc.tile_pool(name="op", bufs=1) as op,
        ):
            xs = consts.tile([BLK, IB * M], f32)
            nc.sync.dma_start(out=xs, in_=xp[:, :])
            sc = consts.tile([BLK, IB * OBL], f32)
            nc.sync.dma_start(out=sc, in_=ss[:, :])

            for _ in range(iters):
                ps = pp.tile([M, O_LOC], f32)

                for ib in range(IB):
                    w = wp.tile([BLK, O_LOC], f32)
                    nc.sync.dma_start(out=w, in_=wt[ib * BLK : (ib + 1) * BLK, :])
                    ws = wsp.tile([BLK, O_LOC], f32)
                    nc.vector.tensor_mul(
                        out=ws.rearrange("p (ob oc) -> p ob oc", oc=BLK),
                        in0=w.rearrange("p (ob oc) -> p ob oc", oc=BLK),
                        in1=sc[:, ib * OBL : (ib + 1) * OBL].broadcast_to(
                            (BLK, OBL, BLK)
                        ),
                    )
                    for lo in range(0, O_LOC, 512):
                        hi = min(lo + 512, O_LOC)
                        nc.tensor.matmul(
                            ps[:, lo:hi],
                            xs[:, ib * M : (ib + 1) * M],
                            ws[:, lo:hi],
                            start=(ib == 0),
                            stop=(ib == IB - 1),
                        )

                ysb = op.tile([M, O_LOC], f32)
                nc.vector.tensor_copy(out=ysb, in_=ps)
                nc.sync.dma_start(out=y[:, :], in_=ysb)
    nc.compile()
    return nc


def get_nc(iters=1):
    key = ("nc", iters)
    if key not in _CACHE:
        _CACHE[key] = _build_nc(iters)
    return _CACHE[key]


def make_in_maps(x, weight, weight_scale_inv):
    """Host-side shard + layout prep (pure data movement, no arithmetic)."""
    x = np.ascontiguousarray(x, dtype=np.float32)
    weight = np.ascontiguousarray(weight, dtype=np.float32)
    s = np.ascontiguousarray(weight_scale_inv, dtype=np.float32)

    # x packed: xp[p, ib*M + m] = x[m, ib*BLK + p]
    xp = np.ascontiguousarray(
        x.reshape(M, IB, BLK).transpose(2, 1, 0).reshape(BLK, IB * M)
    )

    in_maps = []
    for c in range(NCORES):
        w_c = weight[c * O_LOC : (c + 1) * O_LOC, :]  # [O_LOC, I]
        wt_c = np.ascontiguousarray(w_c.T)  # [I, O_LOC]
        s_c = s[c * OBL : (c + 1) * OBL, :]  # [OBL, IB]
        ss_flat = np.ascontiguousarray(s_c.T).reshape(1, IB * OBL)
        ss_c = np.ascontiguousarray(np.broadcast_to(ss_flat, (BLK, IB * OBL)))
        in_maps.append({"wt": wt_c, "xp": xp, "ss": ss_c})
    return in_maps


def kernel(x, weight, weight_scale_inv):
    from concourse.bass_utils import run_bass_kernel_spmd

    nc = get_nc()
    in_maps = make_in_maps(x, weight, weight_scale_inv)
    res = run_bass_kernel_spmd(nc, in_maps, list(range(NCORES)))
    outs = [res.results[c]["y"] for c in range(NCORES)]
    return np.ascontiguousarray(np.concatenate(outs, axis=1), dtype=np.float32)
